# revision 47
# baseline (speedup 1.0000x reference)
"""Trainium2 Bass kernel for 3-layer GraphSAGE (nn_MCHCGraphSage).

Strategy (8 NeuronCores, SPMD single program):
  - Destination-sharded edges: core k owns dst nodes [k*6250, (k+1)*6250).
  - Features live in HBM as 256B rows in "split-slab address" space
    (_addr): each core's slab is stored [windows 0..39 | 22 zero pad rows |
    windows 40..48] so the inter-layer AllGather goes out in two contiguous
    pieces — piece A (rows [0,5142)) fires mid-layer and overlaps the tail
    windows, only piece B (1152 rows) sits on the layer boundary. hext is
    double-buffered (hext0/hext1) so a piece-A write never races the
    previous layer's in-flight gathers.
  - Random x[src] rows are fetched with gpsimd dma_gather (int16 indices)
    spread round-robin over 4 SWDGE queues (the aggregate random-256B
    packet rate ~4 ns/packet is the kernel's bottleneck). int16 range
    forces a two-section split: section A gathers rows [0, 32768),
    section B rows [BBASE, TOTROW) (base offset BBASE).
  - Segmented mean via two PE matmul levels over dst-sorted, degree-padded
    (multiple of 4) edge slots:
      level 1: 8 wide matmuls per window (constant block-ones lhsT
               [128, 32], one per PE row-quadrant x section) over
               run-contiguous chunk slices; never-written PSUM cells are
               zero-filled once (first psA-pool rotation) and stay zero.
      level 2: host-built one-hot [128 groups, 128 dst] (bf16) with the
               1/deg mean scale folded into its values, accumulated in
               PSUM; one PSUM->SBUF bf16 cast per window (ACT/DVE
               alternating).
  - Dense part per window, node-major: y = meanT.T @ Wl + hselfT.T @ Ws_ext
    (bias folded as an extra ones-row of hselfT, zeroed on pad columns so
    pad slab rows compute to exactly 0), ReLU on ACT, DMA the [128, 64]
    node-major block straight to the own slab; PE-transpose to keep the
    feature-major self slab for the next layer.
"""

import os
import sys

import numpy as np

for _p in ("/opt/trn_rl_repo", "/root/.axon_site/_ro/trn_rl_repo"):
    if os.path.isdir(_p) and _p not in sys.path:
        sys.path.append(_p)

import ml_dtypes  # noqa: E402

N = 50000
D = 64
NCORES = 8
SLAB = 6250
PSLAB = 6272
WIN = 128
NW = PSLAB // WIN  # 49
# Split-slab layout: the per-core slab is stored as
#   [windows 0..SPLIT_W-1 (H1 rows) | 22 zero pad rows | windows SPLIT_W..48]
# so the inter-layer AllGather can go out in two contiguous pieces: piece A
# (rows [0, H1P)) fires once windows < SPLIT_W are done and overlaps the
# tail windows' compute; only piece B sits on the layer boundary.
SPLIT_W = 40
H1 = SPLIT_W * WIN  # 5120
PADN = PSLAB - SLAB  # 22
H1P = H1 + PADN  # 5142 (piece-A rows per core, incl. always-zero pad)
H2 = PSLAB - H1  # 1152 (piece-B rows per core)
PSLAB2 = H1P + H2  # 6294 stored slab rows per core
TOTROW = NCORES * PSLAB2  # 50352
BASE_B = NCORES * H1P  # 41136, start of piece-B region in hext
BBASE = TOTROW - 32768  # 17584, B-section base row
APAD_ROW = H1  # row 5120 (core 0 piece-A pad) is always zero
BPAD_ROW = BASE_B + (SLAB - H1)  # core 0's s=6250 pad row, always zero
BW = 4  # windows per gather batch

_NC_CACHE = {}
LAST_RESULTS = None  # test harness introspection (exec_time_ns, profile)


def _addr(n):
    s = n % SLAB
    k = n // SLAB
    return np.where(s < H1, k * H1P + s, BASE_B + k * H2 + (s - H1))


def _srow(w):
    """Stored slab row of window w's first node."""
    return w * WIN if w < SPLIT_W else H1P + (w - SPLIT_W) * WIN


def _run_split(nch_a, nch_b):
    """Assign the NCH chunks of a window to 4 PE row-quadrants in
    contiguous runs: A chunks split [nA0..nA3], then B chunks [nB0..nB3].
    Quadrant r holds A-run r at col-blocks [0, nA[r]) and B-run r at
    [nA[r], nA[r]+nB[r])."""
    nA = np.array([(nch_a + 3 - r) // 4 for r in range(4)])
    nB = np.array([(nch_b + 3 - r) // 4 for r in range(4)])
    aoff = np.concatenate([[0], np.cumsum(nA)]).astype(np.int64)
    boff = np.concatenate([[0], np.cumsum(nB)]).astype(np.int64)
    runmapA = np.repeat(np.arange(4), nA)
    runmapB = np.repeat(np.arange(4), nB)
    nblk = max(a + b for a, b in zip(nA, nB))
    return nA, nB, aoff, boff, runmapA, runmapB, nblk


def _pack(x, edge_index, scale, use_bf16):
    """Host-side packing (unique-src variant).

    Per (core, window, section) the gather stream holds each DISTINCT src
    address once (sorted — helps HBM locality); the expansion edge->dst is
    a streamed [128 slots, 128 dst] matrix `au` per chunk whose values are
    multiplicity x 1/deg[dst].  win = sum_c gathered_c^T @ au_c directly
    yields the mean, with no intermediate grouping levels.
    """
    src = np.asarray(edge_index[0], dtype=np.int64)
    dst = np.asarray(edge_index[1], dtype=np.int64)
    addr_e = _addr(src)

    # pass 1: global chunk counts from per-(core,window,section) uniques
    nch_a = 0
    nch_b = 0
    per_core = []
    for k in range(NCORES):
        sel = (dst >= k * SLAB) & (dst < (k + 1) * SLAB)
        d_k = dst[sel] - k * SLAB
        a_k = addr_e[sel]
        isA = a_k <= 32767
        w_k = d_k // WIN
        for sec, m in ((0, isA), (1, ~isA)):
            key = w_k[m] * np.int64(TOTROW) + a_k[m]
            uniq = np.unique(key)
            wcnt = np.bincount((uniq // TOTROW).astype(np.int64), minlength=NW)
            mx = int(wcnt.max()) if len(uniq) else 0
            if sec == 0:
                nch_a = max(nch_a, (mx + 127) // 128)
            else:
                nch_b = max(nch_b, (mx + 127) // 128)
        per_core.append((d_k, a_k, isA))

    S_A = nch_a * 128
    S_B = nch_b * 128
    NCHU = nch_a + nch_b
    fdt = ml_dtypes.bfloat16 if use_bf16 else np.float32
    ROW = 128 if use_bf16 else 64

    # xext: node features in split-slab address space, same for all cores
    xext = np.zeros((TOTROW, ROW), dtype=fdt)
    rows = _addr(np.arange(N))
    xext[rows, :D] = x.astype(fdt)

    cores = []
    for k in range(NCORES):
        d_k, a_k, isA = per_core[k]
        w_k = d_k // WIN
        streamA = np.full(NW * S_A, APAD_ROW, dtype=np.int64)
        streamB = np.full(NW * S_B, BPAD_ROW - BBASE, dtype=np.int64)
        au = np.zeros((128, NW * NCHU * 128), dtype=np.float32)
        for sec in (0, 1):
            m = isA if sec == 0 else ~isA
            aa = a_k[m]
            dd = d_k[m]
            ww = w_k[m]
            if len(aa) == 0:
                continue
            base = 0 if sec == 0 else BBASE
            S = S_A if sec == 0 else S_B
            stream = streamA if sec == 0 else streamB
            key = ww * np.int64(TOTROW) + aa
            uniq, inv = np.unique(key, return_inverse=True)
            uw = (uniq // TOTROW).astype(np.int64)
            wstart = np.searchsorted(uw, np.arange(NW))
            slot_of_uniq = np.arange(len(uniq)) - wstart[uw]
            assert slot_of_uniq.max() < S
            stream[uw * S + slot_of_uniq] = (uniq % TOTROW) - base
            es = slot_of_uniq[inv]  # per-edge slot within window-section
            chunk = es // 128 + (0 if sec == 0 else nch_a)
            cols = (ww * NCHU + chunk) * 128 + dd % WIN
            np.add.at(au, (es % 128, cols), scale[k * SLAB + dd])
        assert streamA.max() <= 32767 and streamB.max() <= 32767
        assert streamA.min() >= 0 and streamB.min() >= 0

        stream = np.concatenate([streamA, streamB]).astype(np.int16)
        idx16 = stream.reshape(-1, 16).T.copy()  # [16, T/16]
        idx = np.tile(idx16, (8, 1))  # replicate for 8 gpsimd cores

        xselfT = np.zeros((D + 1, PSLAB), dtype=fdt)
        xselfT[:D, :SLAB] = x[k * SLAB : (k + 1) * SLAB].T.astype(fdt)
        xselfT[D, :SLAB] = 1.0  # bias row; pad columns stay 0 -> relu(0)=0

        cores.append(
            {
                "idx": idx,
                "au": au.astype(ml_dtypes.bfloat16),
                "xselfT": xselfT,
            }
        )

    return nch_a, nch_b, NCHU, xext, cores


def _build_nc(nch_a, nch_b, nblk, use_bf16):
    import concourse.bacc as bacc
    import concourse.tile as tile
    import concourse.mybir as mybir

    dt = mybir.dt
    fdt = dt.bfloat16 if use_bf16 else dt.float32
    ROW = 128 if use_bf16 else 64
    NCHU = nch_a + nch_b
    assert nblk == NCHU
    S_A = nch_a * 128
    S_B = nch_b * 128
    T_A = NW * S_A
    T_B = NW * S_B

    nqueues = int(os.environ.get("SAGE_QUEUES", "4"))
    use_prep = os.environ.get("SAGE_PREP", "") == "1"
    nc = bacc.Bacc(None, num_devices=NCORES, num_swdge_queues=nqueues)

    xext_d = nc.dram_tensor("xext", [TOTROW, ROW], fdt, kind="ExternalInput")
    idx_d = nc.dram_tensor(
        "idx", [128, (T_A + T_B) // 16], dt.int16, kind="ExternalInput"
    )
    au_d = nc.dram_tensor(
        "au", [128, NW * NCHU * 128], dt.bfloat16, kind="ExternalInput"
    )
    xsT_d = nc.dram_tensor("xselfT", [D + 1, PSLAB], fdt, kind="ExternalInput")
    ident_d = nc.dram_tensor("ident", [WIN, WIN], fdt, kind="ExternalInput")
    w_d = {}
    for l, m in ((0, D), (1, D), (2, 1)):
        w_d[f"wl{l}"] = nc.dram_tensor(f"wl{l}", [D, m], fdt, kind="ExternalInput")
        w_d[f"ws{l}"] = nc.dram_tensor(
            f"ws{l}", [D + 1, m], fdt, kind="ExternalInput"
        )
    out_d = nc.dram_tensor("out", [PSLAB, 1], dt.float32, kind="ExternalOutput")

    # double-buffered so layer L+1's allgather (piece A, fired mid-layer)
    # never overwrites the table layer L's late gathers are still reading
    hext_ds = [
        nc.dram_tensor(f"hext{i}", [TOTROW, ROW], fdt, addr_space="Shared")
        for i in range(2)
    ]
    slab_d = nc.dram_tensor("slab", [PSLAB2, ROW], fdt)

    bw_env = int(os.environ.get("SAGE_BW", "1"))
    batches = []
    w0 = 0
    while w0 < NW:
        bw = min(bw_env, NW - w0)
        batches.append((w0, bw))
        w0 += bw
    n_layers = int(os.environ.get("SAGE_LAYERS", "3"))
    n_batch_lim = int(os.environ.get("SAGE_BATCHES", str(len(batches))))
    batches = batches[:n_batch_lim]
    no_cc = os.environ.get("SAGE_NOCC", "") == "1"

    with tile.TileContext(nc) as tc:
        with (
            tc.tile_pool(name="const", bufs=1) as cpool,
            tc.tile_pool(
                name="gpool", bufs=int(os.environ.get("SAGE_GBUFS", "8"))
            ) as gpool,
            tc.tile_pool(
                name="spool", bufs=int(os.environ.get("SAGE_SPOOL", "4"))
            ) as spool,
            tc.tile_pool(
                name="apool", bufs=int(os.environ.get("SAGE_ABUFS", "3"))
            ) as apool,
            tc.tile_pool(
                name="psB", bufs=int(os.environ.get("SAGE_PSB", "3")),
                space="PSUM",
            ) as psB,
            tc.tile_pool(name="psC", bufs=2, space="PSUM") as psC,
        ):
            gsems = (
                [nc.alloc_semaphore(f"gsem{q}") for q in range(nqueues)]
                if use_prep else None
            )
            idx_sb = cpool.tile([128, (T_A + T_B) // 16], dt.int16, tag="idx")
            ident_sb = cpool.tile([WIN, WIN], fdt, tag="ident")
            hs = [cpool.tile([D + 1, PSLAB], fdt, tag=f"hs{i}", name=f"hs{i}")
                  for i in range(3)]
            w_sb = {}
            for l, m in ((0, D), (1, D), (2, 1)):
                w_sb[f"wl{l}"] = cpool.tile([D, m], fdt, tag=f"wl{l}",
                                            name=f"wl{l}")
                w_sb[f"ws{l}"] = cpool.tile([D + 1, m], fdt, tag=f"ws{l}",
                                            name=f"ws{l}")
            zpad_sb = cpool.tile([PADN, ROW], fdt, tag="zpad")

            nc.sync.dma_start(idx_sb[:], idx_d[:])
            nc.sync.dma_start(ident_sb[:], ident_d[:])
            nc.sync.dma_start(hs[0][:], xsT_d[:])
            for l in range(3):
                nc.sync.dma_start(w_sb[f"wl{l}"][:], w_d[f"wl{l}"][:])
                nc.sync.dma_start(w_sb[f"ws{l}"][:], w_d[f"ws{l}"][:])
            nc.vector.memset(zpad_sb[:], 0.0)
            # bias only on real-node columns: pad columns then compute to
            # exactly 0 (relu(0)), so the slab pad rows need no re-zeroing
            nc.vector.memset(hs[1][D : D + 1, 0:SLAB], 1.0)
            nc.vector.memset(hs[1][D : D + 1, SLAB:PSLAB], 0.0)
            nc.vector.memset(hs[2][D : D + 1, 0:SLAB], 1.0)
            nc.vector.memset(hs[2][D : D + 1, SLAB:PSLAB], 0.0)
            # piece-A pad rows of the slab: zeroed once, never written again
            nc.sync.dma_start(slab_d[H1:H1P, :], zpad_sb[:])

            import contextlib
            reps = int(os.environ.get("SAGE_REPS", "1"))
            rep_cm = (tc.For_i(0, reps, 1, name="reploop")
                      if reps > 1 else contextlib.nullcontext())
            with rep_cm:
                for layer in range(n_layers):
                    src_t = xext_d if layer == 0 else hext_ds[layer - 1]
                    hself = hs[layer]
                    wl_t = w_sb[f"wl{layer}"]
                    ws_t = w_sb[f"ws{layer}"]
                    m_out = 1 if layer == 2 else D

                    for bi, (w0, bw) in enumerate(batches):
                        gA = gpool.tile([128, bw * nch_a, ROW], fdt, tag="gA")
                        gB = gpool.tile([128, bw * nch_b, ROW], fdt, tag="gB")
                        numA = bw * S_A
                        numB = bw * S_B
                        a0 = w0 * S_A // 16
                        b0c = (T_A + w0 * S_B) // 16
                        qA = (2 * bi) % nqueues
                        qB = (2 * bi + 1) % nqueues
                        if use_prep:
                            nc.gpsimd.dma_gather(
                                gA[:], src_t[:],
                                idx_sb[:, a0 : a0 + numA // 16],
                                numA, numA, ROW,
                                single_packet=False,
                                queue_num=qA,
                                prepare_only=True, sem=gsems[qA],
                            )
                            nc.gpsimd.trigger_dma(count=None, queue_num=qA)
                            nc.gpsimd.dma_gather(
                                gB[:], src_t[BBASE:, :],
                                idx_sb[:, b0c : b0c + numB // 16],
                                numB, numB, ROW,
                                single_packet=False,
                                queue_num=qB,
                                prepare_only=True, sem=gsems[qB],
                            )
                            nc.gpsimd.trigger_dma(count=None, queue_num=qB)
                        else:
                            nc.gpsimd.dma_gather(
                                gA[:], src_t[:],
                                idx_sb[:, a0 : a0 + numA // 16],
                                numA, numA, ROW,
                                single_packet=False,
                                queue_num=qA,
                            )
                            nc.gpsimd.dma_gather(
                                gB[:], src_t[BBASE:, :],
                                idx_sb[:, b0c : b0c + numB // 16],
                                numB, numB, ROW,
                                single_packet=False,
                                queue_num=qB,
                            )

                        # streamed expansion matrix for this batch's windows
                        au_sb = apool.tile([128, bw * NCHU * 128], dt.bfloat16,
                                           tag="au")
                        nc.sync.dma_start(
                            au_sb[:],
                            au_d[:, w0 * NCHU * 128 : (w0 + bw) * NCHU * 128],
                        )

                        stage = int(os.environ.get("SAGE_STAGE", "9"))
                        for wi in range(bw):
                            if stage < 1:
                                break
                            w = w0 + wi
                            # meanT[f, dst] = sum_c gathered_c^T @ au_c
                            # (multiplicity x 1/deg folded into au values)
                            win_ps = psB.tile([D, WIN], dt.float32, tag="winps")
                            for c in range(NCHU):
                                if c < nch_a:
                                    lhsT = gA[:, wi * nch_a + c, 0:D]
                                else:
                                    lhsT = gB[:, wi * nch_b + (c - nch_a), 0:D]
                                oc = (wi * NCHU + c) * 128
                                nc.tensor.matmul(
                                    win_ps[:], lhsT,
                                    au_sb[:, oc : oc + 128],
                                    start=(c == 0), stop=(c == NCHU - 1),
                                )
                            if stage < 4:
                                continue
                            mean_sb = spool.tile([D, WIN], fdt, tag="mean")
                            if w % 2 == 0:
                                nc.vector.tensor_copy(mean_sb[:], win_ps[:])
                            else:
                                nc.scalar.activation(
                                    mean_sb[:], win_ps[:],
                                    mybir.ActivationFunctionType.Copy,
                                )
                            # dense, node-major: y = meanT.T@Wl + hselfT.T@Ws_ext
                            y_ps = psC.tile([WIN, m_out], dt.float32, tag="ypsum")
                            nc.tensor.matmul(y_ps[:], mean_sb[:], wl_t[:],
                                             start=True, stop=False)
                            nc.tensor.matmul(y_ps[:],
                                             hself[:, w * WIN : (w + 1) * WIN],
                                             ws_t[:], start=False, stop=True)
                            if layer < 2:
                                hn_sb = spool.tile([WIN, D], fdt, tag="hn")
                                nc.scalar.activation(
                                    hn_sb[:], y_ps[:],
                                    mybir.ActivationFunctionType.Relu,
                                )
                                sr = _srow(w)
                                nc.sync.dma_start(
                                    slab_d[sr : sr + WIN, 0:D], hn_sb[:]
                                )
                                t_ps = psB.tile([D, WIN], fdt, tag="tps",
                                                name="t_ps")
                                nc.tensor.transpose(t_ps[:], hn_sb[:], ident_sb[:])
                                nc.vector.tensor_copy(
                                    hs[layer + 1][0:D, w * WIN : (w + 1) * WIN],
                                    t_ps[:],
                                )
                            else:
                                y_sb = spool.tile([WIN, 1], dt.float32, tag="ysb")
                                nc.scalar.activation(
                                    y_sb[:], y_ps[:],
                                    mybir.ActivationFunctionType.Relu,
                                )
                                nc.sync.dma_start(
                                    out_d[w * WIN : (w + 1) * WIN, :], y_sb[:]
                                )

                        if (layer < 2 and layer < n_layers - 1 and not no_cc
                                and w0 + bw == SPLIT_W):
                            # piece A: windows [0, SPLIT_W) + zero pad rows;
                            # overlaps the remaining windows' gathers/compute
                            nc.gpsimd.collective_compute(
                                "AllGather",
                                mybir.AluOpType.bypass,
                                replica_groups=[list(range(NCORES))],
                                ins=[slab_d[0:H1P]],
                                outs=[hext_ds[layer][0 : NCORES * H1P]],
                            )

                    if layer < 2 and layer < n_layers - 1 and not no_cc:
                        nc.gpsimd.collective_compute(
                            "AllGather",
                            mybir.AluOpType.bypass,
                            replica_groups=[list(range(NCORES))],
                            ins=[slab_d[H1P:PSLAB2]],
                            outs=[hext_ds[layer][BASE_B:TOTROW]],
                        )

    nc.compile()
    return nc


def kernel(**inputs):
    x = np.asarray(inputs["x"], dtype=np.float32)
    edge_index = np.asarray(inputs["edge_index"])
    use_bf16 = os.environ.get("SAGE_F32", "") != "1"

    deg = np.bincount(np.asarray(edge_index[1], dtype=np.int64), minlength=N)
    scale = np.where(deg > 0, 1.0 / np.maximum(deg, 1), 0.0).astype(np.float32)

    nch_a, nch_b, nblk, xext, cores = _pack(x, edge_index, scale, use_bf16)

    key = (nch_a, nch_b, nblk, use_bf16)
    if key not in _NC_CACHE:
        _NC_CACHE[key] = _build_nc(nch_a, nch_b, nblk, use_bf16)
    nc = _NC_CACHE[key]

    fdt = ml_dtypes.bfloat16 if use_bf16 else np.float32
    ident = np.eye(WIN, dtype=fdt)

    common = {
        "xext": xext,
        "ident": ident,
    }
    for l in range(3):
        common[f"wl{l}"] = np.asarray(inputs[f"Wl{l}"]).astype(fdt)
        wse = np.concatenate(
            [
                np.asarray(inputs[f"Ws{l}"], np.float32),
                (np.asarray(inputs[f"bl{l}"], np.float32)
                 + np.asarray(inputs[f"bs{l}"], np.float32)).reshape(1, -1),
            ],
            axis=0,
        )
        common[f"ws{l}"] = wse.astype(fdt)

    in_maps = []
    for k in range(NCORES):
        m = dict(common)
        m.update(cores[k])
        in_maps.append(m)

    from concourse.bass_utils import run_bass_kernel_spmd

    res = run_bass_kernel_spmd(nc, in_maps, core_ids=list(range(NCORES)))
    global LAST_RESULTS
    LAST_RESULTS = res
    outs = [np.asarray(res.results[k]["out"]).reshape(-1)[:SLAB]
            for k in range(NCORES)]
    return np.concatenate(outs).reshape(N, 1).astype(np.float32)


if __name__ == "__main__":
    pass

